# revision 25
# baseline (speedup 1.0000x reference)
"""BFP-quantized linear layer (BFLinear) for Trainium2, 8-core data-parallel.

Computes: out = bfp_q(x, 8, 16) @ bfp_q(w, 8, 16).T + bias
  where bfp_q groups 16 contiguous elements along the feature axis, shares
  exponent e = floor(log2(max|g|)), rounds mantissas to `bit` bits (RNE) and
  clips to [-2^(bit-1), 2^(bit-1)-1].

Math on-device (exact vs the jax reference, up to fp32 matmul assoc.):
  gmax  = max|group|                      (DVE reduce, abs)
  e     = max(exponent_field(gmax), 1)    (int ops on bitcast)
  scale = 2^(e-127-(bit-1)) ; inv = 2^((bit-1)-(e-127))   (bit assembly)
  v     = x * inv                         (exact: power-of-2 scaling)
  t     = clamp(v, lo', hi')              (hi' = nextafter(qmax+0.5, 0), etc.)
  r     = (t + 1.5*2^23) - 1.5*2^23       (exact RNE integer round)
  xq    = r * scale                       (exact, bf16-representable)
Then out = xq @ wq.T + bias via bf16 TensorE matmuls accumulated in fp32 PSUM;
bias is seeded into PSUM by a K=1 fp32 matmul (ones ⊗ bias).

Sharding: rows of x split evenly across 8 NeuronCores; weight/bias replicated.
Quantization groups lie along K (feature) so row sharding never splits one.
"""

import os
import sys

import numpy as np

for _p in ("/opt/trn_rl_repo",):
    if _p not in sys.path and os.path.isdir(_p):
        sys.path.append(_p)

N_CORES = 8

# engine assignment knobs (tuned on HW): each is 'v' (DVE), 'g' (GPSIMD),
# 's' (ACT/scalar, round only)
ENG_CFG = {
    "reduce": "v",
    "smalls": "v",
    "mult": "v",
    "clamp": "g",
    "round": "v",
    "scalemult": "v",
}

_CACHE = {}


def _engine(nc, which):
    import concourse.bass as bass  # noqa: F401

    return {"v": nc.vector, "g": nc.gpsimd, "s": nc.scalar}[which]


def _bcast_group_ap(t, G, sz):
    """AP reading tile t[P, G] as [P, G, sz] with the last dim broadcast."""
    import concourse.bass as bass

    ap = t.ap.copy()
    ap.append([0, sz])
    return bass.AP(tensor=t.tensor, offset=t.offset, ap=ap)


def _quantize(nc, pools, xt, F, bit, sz, out_bf16, cfg):
    """Emit BFP quantization of xt [128, F] f32 -> out_bf16 [128, F] bf16.

    pow2e  = 2^e (exponent of clamped group absmax, mantissa masked off)
    recip2 = 2^(1-e)  (exponent field negated via XOR)
    v      = (x * 2^(bit-2)) * recip2      == x * 2^((bit-1)-e)   (exact)
    t      = clamp(v, lo', hi')            (pre-round clip, equivalent)
    r      = (t + 1.5*2^23) - 1.5*2^23     (RNE integer round)
    xq     = (r * 2^-(bit-1)) * pow2e      == r * 2^(e-(bit-1))   (exact)
    """
    import concourse.mybir as mybir

    f32 = mybir.dt.float32
    i32 = mybir.dt.int32
    P = 128
    G = F // sz
    qmax = float(2 ** (bit - 1) - 1)
    hi = float(np.nextafter(np.float32(qmax + 0.5), np.float32(0.0)))
    lo = float(np.nextafter(np.float32(-qmax - 1.5), np.float32(0.0)))
    C = float(np.float32(1.5 * 2.0**23))
    # clamp for all-zero / subnormal groups; 2^-119 keeps inv = 2^(bit-1-e)
    # finite after the 2^(bit-2) pre-scale. Never reached by real data.
    FLT_MIN = float(2.0**-119)
    EXPMASK = 0x7F800000

    ev = _engine(nc, cfg["reduce"])
    es = _engine(nc, cfg["smalls"])
    em = _engine(nc, cfg["mult"])
    ec = _engine(nc, cfg["clamp"])
    er = _engine(nc, cfg["round"])
    ex = _engine(nc, cfg["scalemult"])

    gmax = pools["g1"].tile([P, G], f32, tag="gmax")
    ev.tensor_reduce(
        out=gmax,
        in_=xt.rearrange("p (g s) -> p g s", s=sz),
        axis=mybir.AxisListType.X,
        op=mybir.AluOpType.max,
        apply_absolute_value=True,
    )
    gmc = pools["g1"].tile([P, G], f32, tag="gmc")
    es.tensor_scalar(
        out=gmc,
        in0=gmax,
        scalar1=FLT_MIN,
        scalar2=None,
        op0=mybir.AluOpType.max,
    )
    pow2e = pools["g1"].tile([P, G], i32, tag="pow2e")
    es.tensor_scalar(
        out=pow2e,
        in0=gmc.bitcast(i32),
        scalar1=EXPMASK,
        scalar2=None,
        op0=mybir.AluOpType.bitwise_and,
    )
    recip2 = pools["g1"].tile([P, G], i32, tag="recip2")
    es.tensor_scalar(
        out=recip2,
        in0=pow2e,
        scalar1=EXPMASK,
        scalar2=None,
        op0=mybir.AluOpType.bitwise_xor,
    )
    # bf16 scale = 2^(e-(bit-1)) (exact power of two), expanded to one value
    # per element by the otherwise-idle GPSIMD so the final multiply runs in
    # the DVE's packed-bf16 2x mode.
    bf16 = mybir.dt.bfloat16
    scale16 = pools["g1"].tile([P, G], bf16, tag="scale16")
    es.tensor_scalar(
        out=scale16,
        in0=pow2e.bitcast(f32),
        scalar1=float(2.0 ** (-(bit - 1))),
        scalar2=None,
        op0=mybir.AluOpType.mult,
    )
    sexp = pools["sexp"].tile([P, F], bf16, tag="sexp")
    nc.gpsimd.tensor_copy(out=sexp, in_=_bcast_group_ap(scale16, G, sz))

    v = pools["v"].tile([P, F], f32, tag="v")
    em.scalar_tensor_tensor(
        out=v,
        in0=xt,
        scalar=float(2.0 ** (bit - 2)),
        in1=_bcast_group_ap(recip2.bitcast(f32), G, sz),
        op0=mybir.AluOpType.mult,
        op1=mybir.AluOpType.mult,
    )
    t = pools["t"].tile([P, F], f32, tag="t")
    ec.tensor_scalar(
        out=t,
        in0=v,
        scalar1=hi,
        scalar2=lo,
        op0=mybir.AluOpType.min,
        op1=mybir.AluOpType.max,
    )
    r = pools["r"].tile([P, F], bf16, tag="r")
    if cfg["round"] == "s":
        u = pools["t"].tile([P, F], f32, tag="u")
        nc.scalar.activation(u, t, mybir.ActivationFunctionType.Copy, bias=C)
        nc.scalar.activation(r, u, mybir.ActivationFunctionType.Copy, bias=-C)
    else:
        er.tensor_scalar(
            out=r,
            in0=t,
            scalar1=C,
            scalar2=C,
            op0=mybir.AluOpType.add,
            op1=mybir.AluOpType.subtract,
        )
    ex.tensor_tensor(
        out=out_bf16,
        in0=r,
        in1=sexp,
        op=mybir.AluOpType.mult,
    )


def _build(nrows, K, O, x_bit, w_bit, x_sz, w_sz, cfg=None):
    import concourse.bacc as bacc
    import concourse.bass as bass  # noqa: F401
    import concourse.mybir as mybir
    import concourse.tile as tile
    from concourse.masks import make_identity

    cfg = dict(ENG_CFG, **(cfg or {}))
    f32 = mybir.dt.float32
    bf16 = mybir.dt.bfloat16

    P = 128
    ROWS_PER_CHUNK = 512
    assert nrows % ROWS_PER_CHUNK == 0
    n_chunks = nrows // ROWS_PER_CHUNK
    F = 4 * K  # free columns per chunk
    KC = K // P  # k-chunks (4)
    OB = O // P  # o-blocks (4)

    nc = bacc.Bacc("TRN2", debug=False)
    x_d = nc.dram_tensor("x", (nrows, K), f32, kind="ExternalInput").ap()
    w_d = nc.dram_tensor("w", (O, K), f32, kind="ExternalInput").ap()
    b_d = nc.dram_tensor("b", (1, O), f32, kind="ExternalInput").ap()
    o_d = nc.dram_tensor("out", (nrows, O), f32, kind="ExternalOutput").ap()

    with tile.TileContext(nc) as tc:
        with (
            tc.tile_pool(name="const", bufs=1) as constp,
            tc.tile_pool(name="wsb", bufs=1) as wsb,
            tc.tile_pool(name="xraw", bufs=3) as xraw,
            tc.tile_pool(name="g1", bufs=3) as g1,
            tc.tile_pool(name="v", bufs=2) as vp,
            tc.tile_pool(name="t", bufs=2) as tp,
            tc.tile_pool(name="r", bufs=2) as rp,
            tc.tile_pool(name="sexp", bufs=2) as sexpp,
            tc.tile_pool(name="xq", bufs=2) as xqp,
            tc.tile_pool(name="xqT", bufs=3) as xqTp,
            tc.tile_pool(name="osb", bufs=3) as osb,
            tc.tile_pool(name="psT", bufs=2, space="PSUM") as psT,
            tc.tile_pool(name="psO", bufs=3, space="PSUM") as psO,
        ):
            pools = {"g1": g1, "v": vp, "t": tp, "r": rp, "sexp": sexpp}

            ident = constp.tile([P, P], bf16)
            make_identity(nc, ident)
            ones2 = constp.tile([2, P], bf16)
            nc.vector.memset(ones2, 1.0)
            bias_sb = constp.tile([1, O], f32)
            nc.sync.dma_start(out=bias_sb, in_=b_d)
            # bias split into bf16 hi + lo so a K=2 bf16 matmul seeds PSUM
            # with fp32-accurate bias (error ~2^-17 relative)
            bhi = constp.tile([1, O], bf16)
            nc.vector.tensor_copy(out=bhi, in_=bias_sb)
            bhi32 = constp.tile([1, O], f32)
            nc.vector.tensor_copy(out=bhi32, in_=bhi)
            blo32 = constp.tile([1, O], f32)
            nc.vector.tensor_tensor(
                out=blo32, in0=bias_sb, in1=bhi32, op=mybir.AluOpType.subtract
            )
            blo = constp.tile([1, O], bf16)
            nc.vector.tensor_copy(out=blo, in_=blo32)
            brow = constp.tile([2, O], bf16)
            nc.sync.dma_start(out=brow[0:1, :], in_=bhi)
            nc.sync.dma_start(out=brow[1:2, :], in_=blo)

            # ---- weights: quantize + transpose, resident ----
            wqT = []
            wq_tiles = []
            for ob in range(OB):
                w_raw = wsb.tile([P, K], f32, tag="w_raw", bufs=OB)
                nc.sync.dma_start(out=w_raw, in_=w_d[ob * P : (ob + 1) * P, :])
                wq = wsb.tile([P, K], bf16, tag="wq", bufs=OB)
                _quantize(nc, pools, w_raw, K, w_bit, w_sz, wq, cfg)
                wq_tiles.append(wq)
            for cp in range(KC // 2):
                ptw = psT.tile([P, 2, O], bf16, tag="ptT")
                for g in range(2):
                    ci = cp * 2 + g
                    for ob in range(OB):
                        nc.tensor.transpose(
                            ptw[:, g, ob * P : (ob + 1) * P],
                            wq_tiles[ob][:, ci * P : (ci + 1) * P],
                            ident,
                        )
                wt = wsb.tile([P, 2, O], bf16, tag=f"wqT{cp}")
                nc.scalar.copy(wt, ptw)
                wqT.extend([wt[:, 0, :], wt[:, 1, :]])

            # ---- main loop over row chunks ----
            for c in range(n_chunks):
                x_raw = xraw.tile([P, 4, K], f32, tag="x_raw")
                src = x_d[c * ROWS_PER_CHUNK : (c + 1) * ROWS_PER_CHUNK, :].rearrange(
                    "(f p) k -> p f k", p=P
                )
                nc.sync.dma_start(out=x_raw, in_=src)
                xq = xqp.tile([P, 4, K], bf16, tag="xq")
                _quantize(
                    nc,
                    pools,
                    x_raw.rearrange("p f k -> p (f k)"),
                    F,
                    x_bit,
                    x_sz,
                    xq.rearrange("p f k -> p (f k)"),
                    cfg,
                )
                for fp in range(2):
                    ptT = psT.tile([P, 2, K], bf16, tag="ptT")
                    for g in range(2):
                        f = fp * 2 + g
                        for ci in range(KC):
                            nc.tensor.transpose(
                                ptT[:, g, ci * P : (ci + 1) * P],
                                xq[:, f, ci * P : (ci + 1) * P],
                                ident,
                            )
                    xqT = xqTp.tile([P, 2, K], bf16, tag="xqT")
                    nc.scalar.copy(xqT, ptT)

                    po = psO.tile([P, 2, O], f32, tag="po")
                    for g in range(2):
                        nc.tensor.matmul(
                            po[:, g, :], lhsT=ones2, rhs=brow, start=True, stop=False
                        )
                        for ci in range(KC):
                            nc.tensor.matmul(
                                po[:, g, :],
                                lhsT=xqT[:, g, ci * P : (ci + 1) * P],
                                rhs=wqT[ci],
                                start=False,
                                stop=(ci == KC - 1),
                            )
                    out_sb = osb.tile([P, 2, O], f32, tag="out_sb")
                    nc.scalar.copy(out_sb, po)
                    r0 = c * ROWS_PER_CHUNK + fp * 2 * P
                    dst = o_d[r0 : r0 + 2 * P, :].rearrange("(f p) k -> p f k", p=P)
                    nc.sync.dma_start(out=dst, in_=out_sb)
    nc.compile()
    return nc


def _get_program(nrows, K, O, x_bit, w_bit, x_sz, w_sz):
    key = (nrows, K, O, x_bit, w_bit, x_sz, w_sz)
    if key not in _CACHE:
        _CACHE[key] = _build(nrows, K, O, x_bit, w_bit, x_sz, w_sz)
    return _CACHE[key]


def kernel(input, weight, bias, i_bit, i_sz, w_bit, w_sz):
    from concourse.bass_utils import run_bass_kernel_spmd

    x = np.ascontiguousarray(np.asarray(input, dtype=np.float32))
    w = np.ascontiguousarray(np.asarray(weight, dtype=np.float32))
    b = np.ascontiguousarray(np.asarray(bias, dtype=np.float32)).reshape(1, -1)
    i_bit, i_sz, w_bit, w_sz = int(i_bit), int(i_sz), int(w_bit), int(w_sz)

    N, K = x.shape
    O = w.shape[0]
    assert N % N_CORES == 0
    shard = N // N_CORES

    nc = _get_program(shard, K, O, i_bit, w_bit, i_sz, w_sz)
    in_maps = [
        {"x": x[i * shard : (i + 1) * shard], "w": w, "b": b} for i in range(N_CORES)
    ]
    res = run_bass_kernel_spmd(nc, in_maps, list(range(N_CORES)))
    out = np.concatenate([r["out"] for r in res.results], axis=0)
    return out.astype(np.float32, copy=False)


# revision 31
# speedup vs baseline: 1.4504x; 1.4504x over previous
"""BFP-quantized linear layer (BFLinear) for Trainium2, 8-core data-parallel.

Computes: out = bfp_q(x, 8, 16) @ bfp_q(w, 8, 16).T + bias
  where bfp_q groups 16 contiguous elements along the feature axis, shares
  exponent e = floor(log2(max|g|)), rounds mantissas to `bit` bits (RNE) and
  clips to [-2^(bit-1), 2^(bit-1)-1].

Math on-device (exact vs the jax reference, up to fp32 matmul assoc.):
  gmax  = max|group|                      (DVE reduce, abs)
  e     = max(exponent_field(gmax), 1)    (int ops on bitcast)
  scale = 2^(e-127-(bit-1)) ; inv = 2^((bit-1)-(e-127))   (bit assembly)
  v     = x * inv                         (exact: power-of-2 scaling)
  t     = clamp(v, lo', hi')              (hi' = nextafter(qmax+0.5, 0), etc.)
  r     = (t + 1.5*2^23) - 1.5*2^23       (exact RNE integer round)
  xq    = r * scale                       (exact, bf16-representable)
Then out = xq @ wq.T + bias via bf16 TensorE matmuls accumulated in fp32 PSUM;
bias is seeded into PSUM by a K=1 fp32 matmul (ones ⊗ bias).

Sharding: rows of x split evenly across 8 NeuronCores; weight/bias replicated.
Quantization groups lie along K (feature) so row sharding never splits one.
"""

import os
import sys

import numpy as np

for _p in ("/opt/trn_rl_repo",):
    if _p not in sys.path and os.path.isdir(_p):
        sys.path.append(_p)

N_CORES = 8

# engine assignment knobs (tuned on HW): each is 'v' (DVE), 'g' (GPSIMD),
# 's' (ACT/scalar, round only)
ENG_CFG = {
    "reduce": "v",
    "smalls": "v",
    "mult": "v",
    "clamp": "g",
    "round": "v",
    "scalemult": "v",
}

_CACHE = {}


def _engine(nc, which):
    import concourse.bass as bass  # noqa: F401

    return {"v": nc.vector, "g": nc.gpsimd, "s": nc.scalar}[which]


def _bcast_group_ap(t, G, sz):
    """AP reading tile t[P, G] as [P, G, sz] with the last dim broadcast."""
    import concourse.bass as bass

    ap = t.ap.copy()
    ap.append([0, sz])
    return bass.AP(tensor=t.tensor, offset=t.offset, ap=ap)


def _quantize(nc, pools, xt, F, bit, sz, out_bf16, cfg):
    """Emit BFP quantization of xt [128, F] f32 -> out_bf16 [128, F] bf16.

    pow2e  = 2^e (exponent of clamped group absmax, mantissa masked off)
    recip2 = 2^(1-e)  (exponent field negated via XOR)
    v      = (x * 2^(bit-2)) * recip2      == x * 2^((bit-1)-e)   (exact)
    t      = clamp(v, lo', hi')            (pre-round clip, equivalent)
    r      = (t + 1.5*2^23) - 1.5*2^23     (RNE integer round)
    xq     = (r * 2^-(bit-1)) * pow2e      == r * 2^(e-(bit-1))   (exact)
    """
    import concourse.mybir as mybir

    f32 = mybir.dt.float32
    i32 = mybir.dt.int32
    P = 128
    G = F // sz
    qmax = float(2 ** (bit - 1) - 1)
    hi = float(np.nextafter(np.float32(qmax + 0.5), np.float32(0.0)))
    lo = float(np.nextafter(np.float32(-qmax - 1.5), np.float32(0.0)))
    C = float(np.float32(1.5 * 2.0**23))
    # clamp for all-zero / subnormal groups; 2^-119 keeps inv = 2^(bit-1-e)
    # finite after the 2^(bit-2) pre-scale. Never reached by real data.
    FLT_MIN = float(2.0**-119)
    EXPMASK = 0x7F800000

    ev = _engine(nc, cfg["reduce"])
    es = _engine(nc, cfg["smalls"])
    em = _engine(nc, cfg["mult"])
    ec = _engine(nc, cfg["clamp"])
    er = _engine(nc, cfg["round"])
    ex = _engine(nc, cfg["scalemult"])

    gmax = pools["g1"].tile([P, G], f32, tag="gmax")
    ev.tensor_reduce(
        out=gmax,
        in_=xt.rearrange("p (g s) -> p g s", s=sz),
        axis=mybir.AxisListType.X,
        op=mybir.AluOpType.max,
        apply_absolute_value=True,
    )
    gmc = pools["g1"].tile([P, G], f32, tag="gmc")
    nc.gpsimd.tensor_scalar(
        out=gmc,
        in0=gmax,
        scalar1=FLT_MIN,
        scalar2=None,
        op0=mybir.AluOpType.max,
    )
    pow2e = pools["g1"].tile([P, G], i32, tag="pow2e")
    es.tensor_scalar(
        out=pow2e,
        in0=gmc.bitcast(i32),
        scalar1=EXPMASK,
        scalar2=None,
        op0=mybir.AluOpType.bitwise_and,
    )
    recip2 = pools["g1"].tile([P, G], i32, tag="recip2")
    es.tensor_scalar(
        out=recip2,
        in0=pow2e,
        scalar1=EXPMASK,
        scalar2=None,
        op0=mybir.AluOpType.bitwise_xor,
    )
    v = pools["v"].tile([P, F], f32, tag="v")
    em.scalar_tensor_tensor(
        out=v,
        in0=xt,
        scalar=float(2.0 ** (bit - 2)),
        in1=_bcast_group_ap(recip2.bitcast(f32), G, sz),
        op0=mybir.AluOpType.mult,
        op1=mybir.AluOpType.mult,
    )
    t = pools["t"].tile([P, F], f32, tag="t")
    ec.tensor_scalar(
        out=t,
        in0=v,
        scalar1=hi,
        scalar2=lo,
        op0=mybir.AluOpType.min,
        op1=mybir.AluOpType.max,
    )
    r = pools["r"].tile([P, F], f32, tag="r")
    if cfg["round"] == "s":
        u = pools["t"].tile([P, F], f32, tag="u")
        nc.scalar.activation(u, t, mybir.ActivationFunctionType.Copy, bias=C)
        nc.scalar.activation(r, u, mybir.ActivationFunctionType.Copy, bias=-C)
    else:
        er.tensor_scalar(
            out=r,
            in0=t,
            scalar1=C,
            scalar2=C,
            op0=mybir.AluOpType.add,
            op1=mybir.AluOpType.subtract,
        )
    ex.scalar_tensor_tensor(
        out=out_bf16,
        in0=r,
        scalar=float(2.0 ** (-(bit - 1))),
        in1=_bcast_group_ap(pow2e.bitcast(f32), G, sz),
        op0=mybir.AluOpType.mult,
        op1=mybir.AluOpType.mult,
    )


def _build(nrows, K, O, x_bit, w_bit, x_sz, w_sz, cfg=None):
    import concourse.bacc as bacc
    import concourse.bass as bass  # noqa: F401
    import concourse.mybir as mybir
    import concourse.tile as tile
    from concourse.masks import make_identity

    cfg = dict(ENG_CFG, **(cfg or {}))
    f32 = mybir.dt.float32
    bf16 = mybir.dt.bfloat16

    P = 128
    ROWS_PER_CHUNK = 512
    assert nrows % ROWS_PER_CHUNK == 0
    n_chunks = nrows // ROWS_PER_CHUNK
    F = 4 * K  # free columns per chunk
    KC = K // P  # k-chunks (4)
    OB = O // P  # o-blocks (4)

    nc = bacc.Bacc("TRN2", debug=False)
    x_d = nc.dram_tensor("x", (nrows, K), f32, kind="ExternalInput").ap()
    w_d = nc.dram_tensor("w", (O, K), f32, kind="ExternalInput").ap()
    b_d = nc.dram_tensor("b", (1, O), f32, kind="ExternalInput").ap()
    o_d = nc.dram_tensor("out", (nrows, O), f32, kind="ExternalOutput").ap()

    with tile.TileContext(nc) as tc:
        with (
            tc.tile_pool(name="const", bufs=1) as constp,
            tc.tile_pool(name="wsb", bufs=1) as wsb,
            tc.tile_pool(name="xraw", bufs=3) as xraw,
            tc.tile_pool(name="g1", bufs=4) as g1,
            tc.tile_pool(name="v", bufs=3) as vp,
            tc.tile_pool(name="t", bufs=3) as tp,
            tc.tile_pool(name="r", bufs=3) as rp,
            tc.tile_pool(name="xq", bufs=3) as xqp,
            tc.tile_pool(name="xqT", bufs=4) as xqTp,
            tc.tile_pool(name="osb", bufs=3) as osb,
            tc.tile_pool(name="psT", bufs=2, space="PSUM") as psT,
            tc.tile_pool(name="psO", bufs=3, space="PSUM") as psO,
        ):
            pools = {"g1": g1, "v": vp, "t": tp, "r": rp}

            ident = constp.tile([P, P], bf16)
            make_identity(nc, ident)
            ones2 = constp.tile([2, P], bf16)
            nc.vector.memset(ones2, 1.0)
            bias_sb = constp.tile([1, O], f32)
            nc.sync.dma_start(out=bias_sb, in_=b_d)
            # bias split into bf16 hi + lo so a K=2 bf16 matmul seeds PSUM
            # with fp32-accurate bias (error ~2^-17 relative)
            bhi = constp.tile([1, O], bf16)
            nc.vector.tensor_copy(out=bhi, in_=bias_sb)
            bhi32 = constp.tile([1, O], f32)
            nc.vector.tensor_copy(out=bhi32, in_=bhi)
            blo32 = constp.tile([1, O], f32)
            nc.vector.tensor_tensor(
                out=blo32, in0=bias_sb, in1=bhi32, op=mybir.AluOpType.subtract
            )
            blo = constp.tile([1, O], bf16)
            nc.vector.tensor_copy(out=blo, in_=blo32)
            brow = constp.tile([2, O], bf16)
            nc.sync.dma_start(out=brow[0:1, :], in_=bhi)
            nc.sync.dma_start(out=brow[1:2, :], in_=blo)

            # ---- weights: quantize + transpose, resident ----
            wqT = []
            wq_tiles = []
            for ob in range(OB):
                w_raw = wsb.tile([P, K], f32, tag="w_raw", bufs=OB)
                nc.sync.dma_start(out=w_raw, in_=w_d[ob * P : (ob + 1) * P, :])
                wq = wsb.tile([P, K], bf16, tag="wq", bufs=OB)
                _quantize(nc, pools, w_raw, K, w_bit, w_sz, wq, cfg)
                wq_tiles.append(wq)
            for cp in range(KC // 2):
                ptw = psT.tile([P, 2, O], bf16, tag="ptT")
                for g in range(2):
                    ci = cp * 2 + g
                    for ob in range(OB):
                        nc.tensor.transpose(
                            ptw[:, g, ob * P : (ob + 1) * P],
                            wq_tiles[ob][:, ci * P : (ci + 1) * P],
                            ident,
                        )
                wt = wsb.tile([P, 2, O], bf16, tag=f"wqT{cp}")
                nc.scalar.copy(wt, ptw)
                wqT.extend([wt[:, 0, :], wt[:, 1, :]])

            # ---- main loop over row chunks ----
            for c in range(n_chunks):
                x_raw = xraw.tile([P, 4, K], f32, tag="x_raw")
                src = x_d[c * ROWS_PER_CHUNK : (c + 1) * ROWS_PER_CHUNK, :].rearrange(
                    "(f p) k -> p f k", p=P
                )
                nc.sync.dma_start(out=x_raw, in_=src)
                xq = xqp.tile([P, 4, K], bf16, tag="xq")
                _quantize(
                    nc,
                    pools,
                    x_raw.rearrange("p f k -> p (f k)"),
                    F,
                    x_bit,
                    x_sz,
                    xq.rearrange("p f k -> p (f k)"),
                    cfg,
                )
                for fp in range(2):
                    ptT = psT.tile([P, 2, K], bf16, tag="ptT")
                    for g in range(2):
                        f = fp * 2 + g
                        for ci in range(KC):
                            nc.tensor.transpose(
                                ptT[:, g, ci * P : (ci + 1) * P],
                                xq[:, f, ci * P : (ci + 1) * P],
                                ident,
                            )
                    xqT = xqTp.tile([P, 2, K], bf16, tag="xqT")
                    nc.scalar.copy(xqT, ptT)

                    po = psO.tile([P, 2, O], f32, tag="po")
                    for g in range(2):
                        nc.tensor.matmul(
                            po[:, g, :], lhsT=ones2, rhs=brow, start=True, stop=False
                        )
                        for ci in range(KC):
                            nc.tensor.matmul(
                                po[:, g, :],
                                lhsT=xqT[:, g, ci * P : (ci + 1) * P],
                                rhs=wqT[ci],
                                start=False,
                                stop=(ci == KC - 1),
                            )
                    out_sb = osb.tile([P, 2, O], f32, tag="out_sb")
                    nc.scalar.copy(out_sb, po)
                    r0 = c * ROWS_PER_CHUNK + fp * 2 * P
                    dst = o_d[r0 : r0 + 2 * P, :].rearrange("(f p) k -> p f k", p=P)
                    nc.sync.dma_start(out=dst, in_=out_sb)
    nc.compile()
    return nc


def _get_program(nrows, K, O, x_bit, w_bit, x_sz, w_sz):
    key = (nrows, K, O, x_bit, w_bit, x_sz, w_sz)
    if key not in _CACHE:
        _CACHE[key] = _build(nrows, K, O, x_bit, w_bit, x_sz, w_sz)
    return _CACHE[key]


def kernel(input, weight, bias, i_bit, i_sz, w_bit, w_sz):
    from concourse.bass_utils import run_bass_kernel_spmd

    x = np.ascontiguousarray(np.asarray(input, dtype=np.float32))
    w = np.ascontiguousarray(np.asarray(weight, dtype=np.float32))
    b = np.ascontiguousarray(np.asarray(bias, dtype=np.float32)).reshape(1, -1)
    i_bit, i_sz, w_bit, w_sz = int(i_bit), int(i_sz), int(w_bit), int(w_sz)

    N, K = x.shape
    O = w.shape[0]
    assert N % N_CORES == 0
    shard = N // N_CORES

    nc = _get_program(shard, K, O, i_bit, w_bit, i_sz, w_sz)
    in_maps = [
        {"x": x[i * shard : (i + 1) * shard], "w": w, "b": b} for i in range(N_CORES)
    ]
    res = run_bass_kernel_spmd(nc, in_maps, list(range(N_CORES)))
    out = np.concatenate([r["out"] for r in res.results], axis=0)
    return out.astype(np.float32, copy=False)
